# revision 32
# baseline (speedup 1.0000x reference)
"""Trainium2 Bass kernel for nn_MultiHeadAttention_87239375716860.

MHA with the reference's quirk: softmax normalizes over the HEADS axis
(score[q,k,b,h], softmax(axis=-1) -> over h), not over keys.

Sharding (no collectives): 8 cores = 4 batches x 2 query-halves.
Core d = 2*b + qc handles batch b, queries [qc*1024, (qc+1)*1024).
Each core projects its batch's full K/V (duplicated between the two
q-half cores) so the softmax-over-heads and the k-contraction are
fully local.

Per-core dataflow (all matmul operands bf16; layouts picked so the only
transpose is a cheap PE-transpose of the attention output):
  qT[e,s] = Ws^T-proj of xq^T (scale 1/8 + bias folded in), stored
            twice with the other head-half zeroed (parity axis) so a
            head-pair's scores are one N=512 matmul at offset 0
  kT[e,s] = Ws^T-proj of xk^T
  v[k,h,dk] = proj of xv^T + bias (DVE add), k on partitions
  per (q-block, k-tile, head-group): s^T[k,q] = kT-chunk @ qT-chunk
  e = exp(s^T) -> bf16;  Z[k,q] = sum_h e (tree split DVE/Pool);
  a = e * (1/Z)  (DVE, in place)
  out[q,e] += a-chunk^T @ v_h   (psum accumulate over k-tiles; a is
            the stationary operand so no zero-padding of v is needed)
  oT[e,q] = PE-transpose(out)  ->  y[q,f] = oT @ Wo^T + b_o -> HBM

The K/V/Q projections are emitted interleaved into the first q-block's
attention iterations so the PE fills softmax bubbles with projection
matmuls instead of idling through a serial phase 1.

HW hazard note: matmuls whose SBUF operands or PSUM outputs sit at
partition offset 64 crash the PE when the offset alternates between
consecutive matmuls.  All matmuls here use full-height (offset-0)
operands; per-head score selection is done by zero-padding the unused
head-half of q.
"""

import numpy as np

SEQ = 2048
BATCH = 4
D = 1024
H = 16
DK = 64
QCH = 1024          # queries per core
NCORES = 8
QB = 256            # q-block size in phase 2
NQB = QCH // QB     # 4
NKT = SEQ // 128    # 16 k-tiles

_CACHE = {}

# schedule/balance knobs (sim-tuned; see test harness sweeps)
CFG = {
    "STAGED": 0,     # softmax emitted in one piece (two-stage split hurt)
    "LAG": 3,        # attn@v emission lag behind scores, in k-tiles
    "EBUFS": 4,      # e-tile ring depth (needs LAG+1 <= EBUFS)
    "XSBUFS": 2,     # x-stream double buffering
    "T1B": "dve",    # engine for second t1 add: "dve" | "pool"
    "T3": "pool",
    "ZF": "dve",
    "KVEX": 1,       # halve K/V projection via pair AllGather exchange
}


def _build_bass():
    """Build + schedule the per-core Bass program (SPMD: same NEFF on all
    8 cores, different input data)."""
    from contextlib import ExitStack

    import concourse.tile as tile
    from concourse import bacc, mybir

    f32 = mybir.dt.float32
    bf16 = mybir.dt.bfloat16
    AF = mybir.ActivationFunctionType

    nc = bacc.Bacc("TRN2", target_bir_lowering=False, debug=False,
                   num_devices=NCORES)

    xqT_d = nc.dram_tensor("xqT", [D, QCH], bf16, kind="ExternalInput").ap()
    KVEX = CFG.get("KVEX", 0)
    kcols = SEQ // 2 if KVEX else SEQ
    xkT_d = nc.dram_tensor("xkT", [D, kcols], bf16, kind="ExternalInput").ap()
    xvT_d = nc.dram_tensor("xvT", [D, kcols], bf16, kind="ExternalInput").ap()
    kst_in = kst_out = vst_in = vst_out = None
    if KVEX:
        kst_in = [nc.dram_tensor(f"kst_in{i}", [128, 8, 512], bf16,
                                 kind="Internal").ap() for i in range(2)]
        kst_out = [nc.dram_tensor(f"kst_out{i}", [2, 128, 8, 512], bf16,
                                  kind="Internal").ap() for i in range(2)]
        vst_in = [nc.dram_tensor(f"vst_in{i}", [128, 4, 1024], bf16,
                                 kind="Internal").ap() for i in range(2)]
        vst_out = [nc.dram_tensor(f"vst_out{i}", [2, 128, 4, 1024], bf16,
                                  kind="Internal").ap() for i in range(2)]
    wsT_d = nc.dram_tensor("wsT", [D, D], bf16, kind="ExternalInput").ap()
    woT_d = nc.dram_tensor("woT", [D, D], bf16, kind="ExternalInput").ap()
    bq_d = nc.dram_tensor("bq", [128, 8], f32, kind="ExternalInput").ap()
    bk_d = nc.dram_tensor("bk", [128, 8], f32, kind="ExternalInput").ap()
    bvb_d = nc.dram_tensor("bvb", [128, D], bf16, kind="ExternalInput").ap()
    bo_d = nc.dram_tensor("bo", [32, D], bf16, kind="ExternalInput").ap()
    idm_d = nc.dram_tensor("idm", [128, 128], bf16, kind="ExternalInput").ap()
    out_d = nc.dram_tensor("out", [QCH, D], f32, kind="ExternalOutput").ap()
    dbg = {}
    if CFG.get("DEBUG"):
        dbg["kT"] = nc.dram_tensor("d_kT", [128, 8 * SEQ], bf16,
                                   kind="ExternalOutput").ap()
        dbg["qT2"] = nc.dram_tensor("d_qT2", [128, 8 * 2 * QCH], bf16,
                                    kind="ExternalOutput").ap()
        dbg["v"] = nc.dram_tensor("d_v", [128, NKT * H * DK], bf16,
                                  kind="ExternalOutput").ap()
        dbg["e0"] = nc.dram_tensor("d_e0", [128, H * QB], bf16,
                                   kind="ExternalOutput").ap()
        dbg["oq0"] = nc.dram_tensor("d_oq0", [128, 2 * 1024], bf16,
                                    kind="ExternalOutput").ap()
        dbg["oT0"] = nc.dram_tensor("d_oT0", [128, 2 * 8 * 128], bf16,
                                    kind="ExternalOutput").ap()

    with tile.TileContext(nc) as tc, ExitStack() as ctx:
        # ---------------- pools ----------------
        persist = ctx.enter_context(tc.tile_pool(name="persist", bufs=1))
        psum_s = ctx.enter_context(tc.tile_pool(name="psum_s", space="PSUM",
                                                bufs=2))
        psum_o = ctx.enter_context(tc.tile_pool(name="psum_o", space="PSUM",
                                                bufs=1))
        work = ctx.enter_context(tc.tile_pool(name="work", bufs=1))

        ws = persist.tile([128, 8, D], bf16, name="ws")
        wo = persist.tile([128, 8, D], bf16, name="wo")
        # qT stored twice with the other head-half zeroed, interleaved
        # [p, chunk, parity, s] so a head-pair's scores are one N=512 matmul.
        qT2 = persist.tile([128, 8, 2, QCH], bf16, name="qT2")
        kT = persist.tile([128, 8, SEQ], bf16, name="kT")
        v = persist.tile([128, NKT, H, DK], bf16, name="v")
        bq_t = persist.tile([128, 8], f32, name="bq_t")
        bk_t = persist.tile([128, 8], f32, name="bk_t")
        bvb_t = persist.tile([128, D], bf16, name="bvb_t")
        # bias-row operands for the K=1-style y-bias matmul, padded to the
        # 32-row PE tile minimum with explicit zeros (rows 1..31 would
        # otherwise contract garbage SBUF partitions).
        bo_t = persist.tile([32, D], bf16, name="bo_t")
        ones_t = persist.tile([32, 128], bf16, name="ones_t")
        id_t = persist.tile([128, 128], bf16, name="id_t")

        for c0 in range(8):
            nc.sync.dma_start(
                ws[:, c0, :],
                wsT_d[c0 * 128:(c0 + 1) * 128, :].rearrange("p e -> p e"))
        nc.sync.dma_start(wo[:], woT_d.rearrange("(c p) e -> p c e", p=128))
        nc.gpsimd.memset(qT2[64:128, :, 0, :], 0.0)
        nc.gpsimd.memset(qT2[0:64, :, 1, :], 0.0)
        nc.sync.dma_start(bq_t[:], bq_d)
        nc.sync.dma_start(bk_t[:], bk_d)
        nc.sync.dma_start(bvb_t[:], bvb_d)
        nc.sync.dma_start(bo_t[:], bo_d)
        nc.gpsimd.memset(ones_t[:], 0.0)
        nc.gpsimd.memset(ones_t[0:1, :], 1.0)
        nc.sync.dma_start(id_t[:], idm_d)

        xs_tiles = {}

        def qproj(sb, c0, c1):
            """Project q columns [sb*512,(sb+1)*512) for chunks c0..c1-1."""
            key = f"q{sb}"
            if key not in xs_tiles:
                t = work.tile([128, 8, 512], bf16, tag="xs", bufs=CFG.get("XSBUFS", 1),
                              name=f"xq{sb}")
                nc.sync.dma_start(
                    t[:], xqT_d[:, sb * 512:(sb + 1) * 512]
                    .rearrange("(c p) s -> p c s", p=128))
                xs_tiles[key] = t
            xq_s = xs_tiles[key]
            for c in range(c0, c1):
                ps = psum_s.tile([128, 1024], f32, tag="s",
                                 name=f"psq{sb}_{c}")
                for dch in range(8):
                    nc.tensor.matmul(
                        ps[:, 0:512],
                        ws[:, dch, c * 128:(c + 1) * 128],
                        xq_s[:, dch, :],
                        start=(dch == 0), stop=(dch == 7))
                nc.scalar.activation(
                    qT2[0:64, c, 0, sb * 512:(sb + 1) * 512],
                    ps[0:64, 0:512],
                    AF.Identity, bias=bq_t[0:64, c:c + 1], scale=0.125)
                nc.scalar.activation(
                    qT2[64:128, c, 1, sb * 512:(sb + 1) * 512],
                    ps[64:128, 0:512],
                    AF.Identity, bias=bq_t[64:128, c:c + 1], scale=0.125)

        def kproj(sb, c0, c1):
            key = f"k{sb}"
            if key not in xs_tiles:
                t = work.tile([128, 8, 512], bf16, tag="xs", bufs=CFG.get("XSBUFS", 1),
                              name=f"xk{sb}")
                nc.sync.dma_start(
                    t[:], xkT_d[:, sb * 512:(sb + 1) * 512]
                    .rearrange("(c p) s -> p c s", p=128))
                xs_tiles[key] = t
            xk_s = xs_tiles[key]
            for c in range(c0, c1):
                ps = psum_s.tile([128, 1024], f32, tag="s",
                                 name=f"psk{sb}_{c}")
                for dch in range(8):
                    nc.tensor.matmul(
                        ps[:, 0:512],
                        ws[:, dch, c * 128:(c + 1) * 128],
                        xk_s[:, dch, :],
                        start=(dch == 0), stop=(dch == 7))
                nc.scalar.activation(
                    kT[:, c, sb * 512:(sb + 1) * 512], ps[:, 0:512],
                    AF.Identity, bias=bk_t[:, c:c + 1])

        def vproj(kch):
            xv_s = work.tile([128, 8, 128], bf16, tag="xv", bufs=CFG.get("XVBUFS", 2),
                             name=f"xv{kch}")
            nc.sync.dma_start(
                xv_s[:], xvT_d[:, kch * 128:(kch + 1) * 128]
                .rearrange("(c p) k -> p c k", p=128))
            ps = psum_s.tile([128, 1024], f32, tag="s", name=f"psv{kch}")
            for eb in range(2):
                for dch in range(8):
                    nc.tensor.matmul(
                        ps[:, eb * 512:(eb + 1) * 512],
                        xv_s[:, dch, :],
                        ws[:, dch, eb * 512:(eb + 1) * 512],
                        start=(dch == 0), stop=(dch == 7))
            nc.vector.tensor_add(
                v.rearrange("p t h k -> p t (h k)")[:, kch, :], ps[:, :],
                bvb_t[:, :])

        PAIRS = [[0, 1], [2, 3], [4, 5], [6, 7]]

        def kx(sb):
            """Exchange own k-projection block sb with the pair partner.
            Slot layout after exchange: kT columns [r*1024 + u] hold rank
            r's own-local tile u//128 (k-order is free: attention sums
            over k)."""
            nc.sync.dma_start(kst_in[sb], kT[:, :, sb * 512:(sb + 1) * 512])
            nc.gpsimd.collective_compute(
                "AllGather", mybir.AluOpType.bypass, replica_groups=PAIRS,
                ins=[kst_in[sb]], outs=[kst_out[sb]])
            for r in range(2):
                stg = work.tile([128, 8, 512], bf16, tag="xs",
                                bufs=CFG.get("XSBUFS", 1), name=f"kstg{sb}{r}")
                nc.sync.dma_start(stg[:], kst_out[sb][r])
                nc.vector.tensor_copy(
                    kT[:, :, r * 1024 + sb * 512:r * 1024 + (sb + 1) * 512],
                    stg[:])

        def vx(vg):
            src = v.rearrange("p t h k -> p t (h k)")[:, 4 * vg:4 * (vg + 1), :]
            nc.sync.dma_start(vst_in[vg], src)
            nc.gpsimd.collective_compute(
                "AllGather", mybir.AluOpType.bypass, replica_groups=PAIRS,
                ins=[vst_in[vg]], outs=[vst_out[vg]])
            vflat = v.rearrange("p t h k -> p t (h k)")
            for r in range(2):
                stg = work.tile([128, 4, 1024], bf16, tag="xs",
                                bufs=CFG.get("XSBUFS", 1), name=f"vstg{vg}{r}")
                nc.sync.dma_start(stg[:], vst_out[vg][r])
                nc.vector.tensor_copy(
                    vflat[:, r * 8 + 4 * vg:r * 8 + 4 * (vg + 1), :],
                    stg[:])

        # startup: enough projections for (qb=0, kt=0); the rest interleave.
        qproj(0, 0, 8)
        kproj(0, 0, 8)
        nc.sync.dma_start(bvb_t[:], bvb_d)
        if KVEX:
            kx(0)
            for t in range(4):
                vproj(t)
            vx(0)
        else:
            vproj(0)
            vproj(1)
        nc.sync.dma_start(wo[:], woT_d.rearrange("(c p) e -> p c e", p=128))
        nc.sync.dma_start(bo_t[:], bo_d)
        nc.sync.dma_start(id_t[:], idm_d)
        nc.gpsimd.memset(ones_t[:], 0.0)
        nc.gpsimd.memset(ones_t[0:1, :], 1.0)

        # ---------------- attention + out-proj ----------------
        LAG = CFG["LAG"]  # attn@v for k-tile kt is emitted after scores(kt+LAG) so
        #          the in-order PE queue never head-of-line blocks on the
        #          cross-engine softmax chain.
        def emit_evac(qb, q0, out_ps):
            # evacuate out[q,e], transpose to oT[e,q], out-project, store
            oq = work.tile([128, 2, 1024], bf16, tag="oq", bufs=1,
                           name=f"oq{qb}")
            for qc in range(2):
                nc.scalar.activation(oq[:, qc, :], out_ps[:, qc, :], AF.Copy)
            tp = psum_s.tile([128, 16, 128], bf16, tag="s", name=f"tp{qb}")
            for qc in range(2):
                for cch in range(8):
                    nc.tensor.transpose(
                        tp[:, qc * 8 + cch, :],
                        oq[:, qc, cch * 128:(cch + 1) * 128], id_t[:])
            oT = work.tile([128, 2, 8, 128], bf16, tag="oT", bufs=1,
                           name=f"oT{qb}")
            nc.vector.tensor_copy(oT[:, 0, :, :], tp[:, 0:8, :])
            nc.vector.tensor_copy(oT[:, 1, :, :], tp[:, 8:16, :])
            if CFG.get("DEBUG") and qb == 0:
                nc.sync.dma_start(dbg["oq0"],
                                  oq.rearrange("p c f -> p (c f)"))
                nc.sync.dma_start(dbg["oT0"],
                                  oT.rearrange("p c t q -> p (c t q)"))
            for qc in range(2):
                yps = psum_s.tile([128, 1024], f32, tag="s",
                                  name=f"yps{qb}_{qc}")
                for fb in range(2):
                    for cch in range(8):
                        nc.tensor.matmul(
                            yps[:, fb * 512:(fb + 1) * 512],
                            oT[:, qc, cch, :],
                            wo[:, cch, fb * 512:(fb + 1) * 512],
                            start=(cch == 0), stop=False)
                    nc.tensor.matmul(
                        yps[:, fb * 512:(fb + 1) * 512], ones_t[:, :],
                        bo_t[:, fb * 512:(fb + 1) * 512],
                        start=False, stop=True)
                y = work.tile([128, 1024], f32, tag="y", bufs=1,
                              name=f"y{qb}_{qc}")
                nc.scalar.activation(y[:], yps[:], AF.Copy)
                nc.sync.dma_start(
                    out_d[q0 + qc * 128: q0 + (qc + 1) * 128, :], y[:])

        pending_evac = []

        for qb in range(NQB):
            q0 = qb * QB
            out_ps = psum_o.tile([128, 2, 1024], f32, tag="ot",
                                 name=f"ot{qb}")
            e_tiles = {}
            t1_tiles = {}

            def stageA(qb, pos, kt, e_tiles=e_tiles, t1_tiles=t1_tiles):
                e = e_tiles[pos]
                t1 = work.tile([128, 2, 4, QB], bf16,
                               tag="t1", bufs=(2 if CFG.get("STAGED", 1) else 1),
                               name=f"t1_{qb}_{kt}")
                t1_tiles[pos] = t1
                nc.vector.tensor_add(t1[:, 0, :, :], e[:, 0:4, :],
                                     e[:, 4:8, :])
                eng1 = nc.gpsimd if CFG["T1B"] == "pool" else nc.vector
                eng1.tensor_add(t1[:, 1, :, :], e[:, 8:12, :],
                                e[:, 12:16, :])

            def stageB(qb, pos, kt, e_tiles=e_tiles, t1_tiles=t1_tiles):
                e = e_tiles[pos]
                t1 = t1_tiles.pop(pos)
                t2 = work.tile([128, 4, QB], bf16, tag="t2",
                               bufs=CFG.get("T2BUFS", 1), name=f"t2_{qb}_{kt}")
                nc.vector.tensor_add(t2[:], t1[:, 0, :, :], t1[:, 1, :, :])
                t3 = work.tile([128, 2, QB], bf16, tag="t3", bufs=1,
                               name=f"t3_{qb}_{kt}")
                eng3 = nc.gpsimd if CFG["T3"] == "pool" else nc.vector
                eng3.tensor_add(t3[:], t2[:, 0:2, :], t2[:, 2:4, :])
                zf = work.tile([128, QB], f32, tag="zf",
                               bufs=CFG.get("ZFBUFS", 2), name=f"zf{qb}_{kt}")
                engz = nc.gpsimd if CFG["ZF"] == "pool" else nc.vector
                engz.tensor_add(zf[:], t3[:, 0, :], t3[:, 1, :])
                rf = work.tile([128, QB], f32, tag="rf", bufs=1,
                               name=f"rf{qb}_{kt}")
                nc.vector.reciprocal_approx_fast(rf[:], zf[:])
                rb = work.tile([128, QB], bf16, tag="rb", bufs=1,
                               name=f"rb{qb}_{kt}")
                nc.vector.tensor_copy(rb[:], rf[:])
                nc.vector.tensor_mul(
                    e[:, :, :], e[:, :, :],
                    rb[:].unsqueeze(1).broadcast_to([128, H, QB]))
                if CFG.get("DEBUG") and qb == 0 and kt == CFG.get("DBG_KT", 0):
                    nc.sync.dma_start(dbg["e0"],
                                      e.rearrange("p h q -> p (h q)"))

            def attnv(pos, kt, qb=qb, out_ps=out_ps, e_tiles=e_tiles):
                e = e_tiles.pop(pos)
                # start/stop once per 2KB PSUM bank: start zeroes the whole
                # bank, so only the first matmul touching a bank may set it.
                for h in range(H):
                    for qc in range(2):
                        nc.tensor.matmul(
                            out_ps[:, qc, h * DK:(h + 1) * DK],
                            e[:, h, qc * 128:(qc + 1) * 128],
                            v[:, kt, h, :],
                            start=(pos == 0 and h % 8 == 0),
                            stop=(pos == NKT - 1 and h % 8 == 7),
                            skip_group_check=True)

            if CFG.get("KVEX", 0):
                # exchanged slot layout: [own 0..3 | partner 0..3 at 8..11 |
                # own 4..7 | partner 4..7].  Process in an order that gives
                # each in-flight exchange ~4 iterations of slack.
                order = [0, 1, 2, 3, 8, 9, 10, 11, 4, 5, 6, 7, 12, 13, 14, 15]
            else:
                order = list(range(NKT))
            for pos in range(NKT):
                kt = order[pos]
                if qb == 0 and CFG.get("KVEX", 0):
                    if pos <= 3:
                        kproj(1, pos * 2, pos * 2 + 2)
                        vproj(pos + 4)
                    elif pos == 4:
                        kx(1)
                    elif pos == 5:
                        vx(1)
                if qb == 0 and not CFG.get("KVEX", 0):
                    blk = kt // 4 + 1
                    if blk < 4:
                        sub = kt % 4
                        kproj(blk, sub * 2, sub * 2 + 2)
                    if kt + 2 < NKT:
                        vproj(kt + 2)
                if qb == 1 and pos <= 3:
                    qproj(1, pos * 2, pos * 2 + 2)
                e = work.tile([128, H, QB], bf16, tag="e", bufs=CFG["EBUFS"],
                              name=f"e{qb}_{kt}")
                e_tiles[pos] = e
                # scores (4 heads = 2 chunk-pairs per psum tile) + exp.
                for hg in range(4):
                    ps = psum_s.tile([128, 1024], f32, tag="s",
                                     name=f"pss{qb}_{kt}_{hg}")
                    for cl in range(2):
                        c = hg * 2 + cl
                        nc.tensor.matmul(
                            ps[:, cl * 512:(cl + 1) * 512],
                            kT[:, c, kt * 128:(kt + 1) * 128],
                            qT2[:, c, :, q0:q0 + QB],
                            start=True, stop=True)
                    nc.scalar.activation(
                        e[:, hg * 4:(hg + 1) * 4, :], ps[:, :], AF.Exp)
                if CFG.get("STAGED", 1):
                    stageA(qb, pos, kt)
                    if pos >= 1:
                        stageB(qb, pos - 1, order[pos - 1])
                else:
                    stageA(qb, pos, kt)
                    stageB(qb, pos, kt)
                if pos >= LAG:
                    attnv(pos - LAG, order[pos - LAG])
                if pos == 1 and pending_evac:
                    pending_evac.pop()()
            if CFG.get("STAGED", 1):
                stageB(qb, NKT - 1, order[NKT - 1])
            for pos in range(NKT - LAG, NKT):
                attnv(pos, order[pos])
            pending_evac.append(lambda qb=qb, q0=q0, out_ps=out_ps:
                                emit_evac(qb, q0, out_ps))
        pending_evac.pop()()
            for qc in range(2):
                nc.scalar.activation(oq[:, qc, :], out_ps[:, qc, :], AF.Copy)
            tp = psum_s.tile([128, 16, 128], bf16, tag="s", name=f"tp{qb}")
            for qc in range(2):
                for cch in range(8):
                    nc.tensor.transpose(
                        tp[:, qc * 8 + cch, :],
                        oq[:, qc, cch * 128:(cch + 1) * 128], id_t[:])
            oT = work.tile([128, 2, 8, 128], bf16, tag="oT", bufs=1,
                           name=f"oT{qb}")
            nc.vector.tensor_copy(oT[:, 0, :, :], tp[:, 0:8, :])
            nc.vector.tensor_copy(oT[:, 1, :, :], tp[:, 8:16, :])
            if CFG.get("DEBUG") and qb == 0:
                nc.sync.dma_start(dbg["oq0"],
                                  oq.rearrange("p c f -> p (c f)"))
                nc.sync.dma_start(dbg["oT0"],
                                  oT.rearrange("p c t q -> p (c t q)"))
            for qc in range(2):
                yps = psum_s.tile([128, 1024], f32, tag="s",
                                  name=f"yps{qb}_{qc}")
                for fb in range(2):
                    for cch in range(8):
                        nc.tensor.matmul(
                            yps[:, fb * 512:(fb + 1) * 512],
                            oT[:, qc, cch, :],
                            wo[:, cch, fb * 512:(fb + 1) * 512],
                            start=(cch == 0), stop=False)
                    nc.tensor.matmul(
                        yps[:, fb * 512:(fb + 1) * 512], ones_t[:, :],
                        bo_t[:, fb * 512:(fb + 1) * 512],
                        start=False, stop=True)
                y = work.tile([128, 1024], f32, tag="y", bufs=1,
                              name=f"y{qb}_{qc}")
                nc.scalar.activation(y[:], yps[:], AF.Copy)
                nc.sync.dma_start(
                    out_d[q0 + qc * 128: q0 + (qc + 1) * 128, :], y[:])

        if CFG.get("DEBUG"):
            nc.sync.dma_start(dbg["kT"], kT.rearrange("p c s -> p (c s)"))
            nc.sync.dma_start(dbg["qT2"],
                              qT2.rearrange("p c t s -> p (c t s)"))
            nc.sync.dma_start(dbg["v"], v.rearrange("p t h k -> p (t h k)"))

    nc.compile()
    return nc


def _get_nc():
    if "nc" not in _CACHE:
        _CACHE["nc"] = _build_bass()
    return _CACHE["nc"]


def _make_in_maps(query, key, value, W_split, b_split, W_o, b_o):
    import ml_dtypes
    bf16 = ml_dtypes.bfloat16

    query = np.asarray(query, np.float32)
    key = np.asarray(key, np.float32)
    value = np.asarray(value, np.float32)
    W_split = np.asarray(W_split, np.float32)
    b_split = np.asarray(b_split, np.float32)
    W_o = np.asarray(W_o, np.float32)
    b_o = np.asarray(b_o, np.float32)

    wsT = np.ascontiguousarray(W_split.T.astype(bf16))
    woT = np.ascontiguousarray(W_o.T.astype(bf16))
    bq = np.ascontiguousarray((b_split / 8.0).reshape(8, 128).T)
    bk = np.ascontiguousarray(b_split.reshape(8, 128).T)
    bvb = np.ascontiguousarray(
        np.broadcast_to(b_split, (128, D)).astype(bf16))
    bo = np.zeros((32, D), bf16)
    bo[0] = b_o.astype(bf16)
    idm = np.eye(128, dtype=bf16)

    if CFG.get("KVEX", 0):
        own_rows = [np.concatenate([np.arange(t * 128, (t + 1) * 128)
                                    for t in range(p, 16, 2)])
                    for p in range(2)]
        kTs = [[np.ascontiguousarray(key[own_rows[p], b, :].T.astype(bf16))
                for p in range(2)] for b in range(BATCH)]
        vTs = [[np.ascontiguousarray(value[own_rows[p], b, :].T.astype(bf16))
                for p in range(2)] for b in range(BATCH)]
    else:
        kTs = [np.ascontiguousarray(key[:, b, :].T.astype(bf16))
               for b in range(BATCH)]
        vTs = [np.ascontiguousarray(value[:, b, :].T.astype(bf16))
               for b in range(BATCH)]
    in_maps = []
    for d in range(NCORES):
        b, qc = d // 2, d % 2
        xqT = np.ascontiguousarray(
            query[qc * QCH:(qc + 1) * QCH, b, :].T.astype(bf16))
        xk = kTs[b][qc] if CFG.get("KVEX", 0) else kTs[b]
        xv = vTs[b][qc] if CFG.get("KVEX", 0) else vTs[b]
        in_maps.append({
            "xqT": xqT, "xkT": xk, "xvT": xv,
            "wsT": wsT, "woT": woT,
            "bq": bq, "bk": bk, "bvb": bvb, "bo": bo, "idm": idm,
        })
    return in_maps


def kernel_with_results(trace=False, **inputs):
    from concourse.bass_utils import run_bass_kernel_spmd

    nc = _get_nc()
    in_maps = _make_in_maps(**inputs)
    last_exc = None
    for _attempt in range(3):
        try:
            res = run_bass_kernel_spmd(nc, in_maps,
                                       core_ids=list(range(NCORES)),
                                       trace=trace)
            break
        except Exception as exc:  # rare transient device fault -> retry
            last_exc = exc
    else:
        raise last_exc
    out = np.empty((SEQ, BATCH, D), np.float32)
    for d in range(NCORES):
        b, qc = d // 2, d % 2
        out[qc * QCH:(qc + 1) * QCH, b, :] = res.results[d]["out"]
    return out, res


def kernel(**inputs):
    out, _ = kernel_with_results(trace=False, **inputs)
    return out


# revision 33
# speedup vs baseline: 1.0751x; 1.0751x over previous
"""Trainium2 Bass kernel for nn_MultiHeadAttention_87239375716860.

MHA with the reference's quirk: softmax normalizes over the HEADS axis
(score[q,k,b,h], softmax(axis=-1) -> over h), not over keys.

Sharding (no collectives): 8 cores = 4 batches x 2 query-halves.
Core d = 2*b + qc handles batch b, queries [qc*1024, (qc+1)*1024).
Each core projects its batch's full K/V (duplicated between the two
q-half cores) so the softmax-over-heads and the k-contraction are
fully local.

Per-core dataflow (all matmul operands bf16; layouts picked so the only
transpose is a cheap PE-transpose of the attention output):
  qT[e,s] = Ws^T-proj of xq^T (scale 1/8 + bias folded in), stored
            twice with the other head-half zeroed (parity axis) so a
            head-pair's scores are one N=512 matmul at offset 0
  kT[e,s] = Ws^T-proj of xk^T
  v[k,h,dk] = proj of xv^T + bias (DVE add), k on partitions
  per (q-block, k-tile, head-group): s^T[k,q] = kT-chunk @ qT-chunk
  e = exp(s^T) -> bf16;  Z[k,q] = sum_h e (tree split DVE/Pool);
  a = e * (1/Z)  (DVE, in place)
  out[q,e] += a-chunk^T @ v_h   (psum accumulate over k-tiles; a is
            the stationary operand so no zero-padding of v is needed)
  oT[e,q] = PE-transpose(out)  ->  y[q,f] = oT @ Wo^T + b_o -> HBM

The K/V/Q projections are emitted interleaved into the first q-block's
attention iterations so the PE fills softmax bubbles with projection
matmuls instead of idling through a serial phase 1.

HW hazard note: matmuls whose SBUF operands or PSUM outputs sit at
partition offset 64 crash the PE when the offset alternates between
consecutive matmuls.  All matmuls here use full-height (offset-0)
operands; per-head score selection is done by zero-padding the unused
head-half of q.
"""

import numpy as np

SEQ = 2048
BATCH = 4
D = 1024
H = 16
DK = 64
QCH = 1024          # queries per core
NCORES = 8
QB = 256            # q-block size in phase 2
NQB = QCH // QB     # 4
NKT = SEQ // 128    # 16 k-tiles

_CACHE = {}

# schedule/balance knobs (sim-tuned; see test harness sweeps)
CFG = {
    "STAGED": 0,     # softmax emitted in one piece (two-stage split hurt)
    "LAG": 3,        # attn@v emission lag behind scores, in k-tiles
    "EBUFS": 4,      # e-tile ring depth (needs LAG+1 <= EBUFS)
    "XSBUFS": 2,     # x-stream double buffering
    "T1B": "dve",    # engine for second t1 add: "dve" | "pool"
    "T3": "pool",
    "ZF": "dve",
}


def _build_bass():
    """Build + schedule the per-core Bass program (SPMD: same NEFF on all
    8 cores, different input data)."""
    from contextlib import ExitStack

    import concourse.tile as tile
    from concourse import bacc, mybir

    f32 = mybir.dt.float32
    bf16 = mybir.dt.bfloat16
    AF = mybir.ActivationFunctionType

    nc = bacc.Bacc("TRN2", target_bir_lowering=False, debug=False,
                   num_devices=NCORES)

    xqT_d = nc.dram_tensor("xqT", [D, QCH], bf16, kind="ExternalInput").ap()
    xkT_d = nc.dram_tensor("xkT", [D, SEQ], bf16, kind="ExternalInput").ap()
    xvT_d = nc.dram_tensor("xvT", [D, SEQ], bf16, kind="ExternalInput").ap()
    wsT_d = nc.dram_tensor("wsT", [D, D], bf16, kind="ExternalInput").ap()
    woT_d = nc.dram_tensor("woT", [D, D], bf16, kind="ExternalInput").ap()
    bq_d = nc.dram_tensor("bq", [128, 8], f32, kind="ExternalInput").ap()
    bk_d = nc.dram_tensor("bk", [128, 8], f32, kind="ExternalInput").ap()
    bvb_d = nc.dram_tensor("bvb", [128, D], bf16, kind="ExternalInput").ap()
    bo_d = nc.dram_tensor("bo", [32, D], bf16, kind="ExternalInput").ap()
    idm_d = nc.dram_tensor("idm", [128, 128], bf16, kind="ExternalInput").ap()
    out_d = nc.dram_tensor("out", [QCH, D], f32, kind="ExternalOutput").ap()
    dbg = {}
    if CFG.get("DEBUG"):
        dbg["kT"] = nc.dram_tensor("d_kT", [128, 8 * SEQ], bf16,
                                   kind="ExternalOutput").ap()
        dbg["qT2"] = nc.dram_tensor("d_qT2", [128, 8 * 2 * QCH], bf16,
                                    kind="ExternalOutput").ap()
        dbg["v"] = nc.dram_tensor("d_v", [128, NKT * H * DK], bf16,
                                  kind="ExternalOutput").ap()
        dbg["e0"] = nc.dram_tensor("d_e0", [128, H * QB], bf16,
                                   kind="ExternalOutput").ap()
        dbg["oq0"] = nc.dram_tensor("d_oq0", [128, 2 * 1024], bf16,
                                    kind="ExternalOutput").ap()
        dbg["oT0"] = nc.dram_tensor("d_oT0", [128, 2 * 8 * 128], bf16,
                                    kind="ExternalOutput").ap()

    with tile.TileContext(nc) as tc, ExitStack() as ctx:
        # ---------------- pools ----------------
        persist = ctx.enter_context(tc.tile_pool(name="persist", bufs=1))
        psum_s = ctx.enter_context(tc.tile_pool(name="psum_s", space="PSUM",
                                                bufs=2))
        psum_o = ctx.enter_context(tc.tile_pool(name="psum_o", space="PSUM",
                                                bufs=1))
        work = ctx.enter_context(tc.tile_pool(name="work", bufs=1))

        ws = persist.tile([128, 8, D], bf16, name="ws")
        wo = persist.tile([128, 8, D], bf16, name="wo")
        # qT stored twice with the other head-half zeroed, interleaved
        # [p, chunk, parity, s] so a head-pair's scores are one N=512 matmul.
        qT2 = persist.tile([128, 8, 2, QCH], bf16, name="qT2")
        kT = persist.tile([128, 8, SEQ], bf16, name="kT")
        v = persist.tile([128, NKT, H, DK], bf16, name="v")
        bq_t = persist.tile([128, 8], f32, name="bq_t")
        bk_t = persist.tile([128, 8], f32, name="bk_t")
        bvb_t = persist.tile([128, D], bf16, name="bvb_t")
        # bias-row operands for the K=1-style y-bias matmul, padded to the
        # 32-row PE tile minimum with explicit zeros (rows 1..31 would
        # otherwise contract garbage SBUF partitions).
        bo_t = persist.tile([32, D], bf16, name="bo_t")
        ones_t = persist.tile([32, 128], bf16, name="ones_t")
        id_t = persist.tile([128, 128], bf16, name="id_t")

        for c0 in range(8):
            nc.sync.dma_start(
                ws[:, c0, :],
                wsT_d[c0 * 128:(c0 + 1) * 128, :].rearrange("p e -> p e"))
        nc.sync.dma_start(wo[:], woT_d.rearrange("(c p) e -> p c e", p=128))
        nc.gpsimd.memset(qT2[64:128, :, 0, :], 0.0)
        nc.gpsimd.memset(qT2[0:64, :, 1, :], 0.0)
        nc.sync.dma_start(bq_t[:], bq_d)
        nc.sync.dma_start(bk_t[:], bk_d)
        nc.sync.dma_start(bvb_t[:], bvb_d)
        nc.sync.dma_start(bo_t[:], bo_d)
        nc.gpsimd.memset(ones_t[:], 0.0)
        nc.gpsimd.memset(ones_t[0:1, :], 1.0)
        nc.sync.dma_start(id_t[:], idm_d)

        xs_tiles = {}

        def qproj(sb, c0, c1):
            """Project q columns [sb*512,(sb+1)*512) for chunks c0..c1-1."""
            key = f"q{sb}"
            if key not in xs_tiles:
                t = work.tile([128, 8, 512], bf16, tag="xs", bufs=CFG.get("XSBUFS", 1),
                              name=f"xq{sb}")
                nc.sync.dma_start(
                    t[:], xqT_d[:, sb * 512:(sb + 1) * 512]
                    .rearrange("(c p) s -> p c s", p=128))
                xs_tiles[key] = t
            xq_s = xs_tiles[key]
            for c in range(c0, c1):
                ps = psum_s.tile([128, 1024], f32, tag="s",
                                 name=f"psq{sb}_{c}")
                for dch in range(8):
                    nc.tensor.matmul(
                        ps[:, 0:512],
                        ws[:, dch, c * 128:(c + 1) * 128],
                        xq_s[:, dch, :],
                        start=(dch == 0), stop=(dch == 7))
                nc.scalar.activation(
                    qT2[0:64, c, 0, sb * 512:(sb + 1) * 512],
                    ps[0:64, 0:512],
                    AF.Identity, bias=bq_t[0:64, c:c + 1], scale=0.125)
                nc.scalar.activation(
                    qT2[64:128, c, 1, sb * 512:(sb + 1) * 512],
                    ps[64:128, 0:512],
                    AF.Identity, bias=bq_t[64:128, c:c + 1], scale=0.125)

        def kproj(sb, c0, c1):
            key = f"k{sb}"
            if key not in xs_tiles:
                t = work.tile([128, 8, 512], bf16, tag="xs", bufs=CFG.get("XSBUFS", 1),
                              name=f"xk{sb}")
                nc.sync.dma_start(
                    t[:], xkT_d[:, sb * 512:(sb + 1) * 512]
                    .rearrange("(c p) s -> p c s", p=128))
                xs_tiles[key] = t
            xk_s = xs_tiles[key]
            for c in range(c0, c1):
                ps = psum_s.tile([128, 1024], f32, tag="s",
                                 name=f"psk{sb}_{c}")
                for dch in range(8):
                    nc.tensor.matmul(
                        ps[:, 0:512],
                        ws[:, dch, c * 128:(c + 1) * 128],
                        xk_s[:, dch, :],
                        start=(dch == 0), stop=(dch == 7))
                nc.scalar.activation(
                    kT[:, c, sb * 512:(sb + 1) * 512], ps[:, 0:512],
                    AF.Identity, bias=bk_t[:, c:c + 1])

        def vproj(kch):
            xv_s = work.tile([128, 8, 128], bf16, tag="xv", bufs=CFG.get("XVBUFS", 2),
                             name=f"xv{kch}")
            nc.sync.dma_start(
                xv_s[:], xvT_d[:, kch * 128:(kch + 1) * 128]
                .rearrange("(c p) k -> p c k", p=128))
            ps = psum_s.tile([128, 1024], f32, tag="s", name=f"psv{kch}")
            for eb in range(2):
                for dch in range(8):
                    nc.tensor.matmul(
                        ps[:, eb * 512:(eb + 1) * 512],
                        xv_s[:, dch, :],
                        ws[:, dch, eb * 512:(eb + 1) * 512],
                        start=(dch == 0), stop=(dch == 7))
            nc.vector.tensor_add(
                v.rearrange("p t h k -> p t (h k)")[:, kch, :], ps[:, :],
                bvb_t[:, :])

        # startup: enough projections for (qb=0, kt=0); the rest interleave.
        qproj(0, 0, 8)
        kproj(0, 0, 8)
        nc.sync.dma_start(bvb_t[:], bvb_d)
        vproj(0)
        vproj(1)
        nc.sync.dma_start(wo[:], woT_d.rearrange("(c p) e -> p c e", p=128))
        nc.sync.dma_start(bo_t[:], bo_d)
        nc.sync.dma_start(id_t[:], idm_d)
        nc.gpsimd.memset(ones_t[:], 0.0)
        nc.gpsimd.memset(ones_t[0:1, :], 1.0)

        # ---------------- attention + out-proj ----------------
        LAG = CFG["LAG"]  # attn@v for k-tile kt is emitted after scores(kt+LAG) so
        #          the in-order PE queue never head-of-line blocks on the
        #          cross-engine softmax chain.
        def emit_evac(qb, q0, out_ps):
            # evacuate out[q,e], transpose to oT[e,q], out-project, store
            oq = work.tile([128, 2, 1024], bf16, tag="oq", bufs=1,
                           name=f"oq{qb}")
            for qc in range(2):
                nc.scalar.activation(oq[:, qc, :], out_ps[:, qc, :], AF.Copy)
            tp = psum_s.tile([128, 16, 128], bf16, tag="s", name=f"tp{qb}")
            for qc in range(2):
                for cch in range(8):
                    nc.tensor.transpose(
                        tp[:, qc * 8 + cch, :],
                        oq[:, qc, cch * 128:(cch + 1) * 128], id_t[:])
            oT = work.tile([128, 2, 8, 128], bf16, tag="oT", bufs=1,
                           name=f"oT{qb}")
            nc.vector.tensor_copy(oT[:, 0, :, :], tp[:, 0:8, :])
            nc.vector.tensor_copy(oT[:, 1, :, :], tp[:, 8:16, :])
            if CFG.get("DEBUG") and qb == 0:
                nc.sync.dma_start(dbg["oq0"],
                                  oq.rearrange("p c f -> p (c f)"))
                nc.sync.dma_start(dbg["oT0"],
                                  oT.rearrange("p c t q -> p (c t q)"))
            for qc in range(2):
                yps = psum_s.tile([128, 1024], f32, tag="s",
                                  name=f"yps{qb}_{qc}")
                for fb in range(2):
                    for cch in range(8):
                        nc.tensor.matmul(
                            yps[:, fb * 512:(fb + 1) * 512],
                            oT[:, qc, cch, :],
                            wo[:, cch, fb * 512:(fb + 1) * 512],
                            start=(cch == 0), stop=False)
                    nc.tensor.matmul(
                        yps[:, fb * 512:(fb + 1) * 512], ones_t[:, :],
                        bo_t[:, fb * 512:(fb + 1) * 512],
                        start=False, stop=True)
                y = work.tile([128, 1024], f32, tag="y", bufs=1,
                              name=f"y{qb}_{qc}")
                nc.scalar.activation(y[:], yps[:], AF.Copy)
                nc.sync.dma_start(
                    out_d[q0 + qc * 128: q0 + (qc + 1) * 128, :], y[:])

        pending_evac = []

        for qb in range(NQB):
            q0 = qb * QB
            out_ps = psum_o.tile([128, 2, 1024], f32, tag="ot",
                                 name=f"ot{qb}")
            e_tiles = {}
            t1_tiles = {}

            def stageA(qb, kt, e_tiles=e_tiles, t1_tiles=t1_tiles):
                e = e_tiles[kt]
                t1 = work.tile([128, 2, 4, QB], bf16,
                               tag="t1", bufs=(2 if CFG.get("STAGED", 1) else 1),
                               name=f"t1_{qb}_{kt}")
                t1_tiles[kt] = t1
                nc.vector.tensor_add(t1[:, 0, :, :], e[:, 0:4, :],
                                     e[:, 4:8, :])
                eng1 = nc.gpsimd if CFG["T1B"] == "pool" else nc.vector
                eng1.tensor_add(t1[:, 1, :, :], e[:, 8:12, :],
                                e[:, 12:16, :])

            def stageB(qb, kt, e_tiles=e_tiles, t1_tiles=t1_tiles):
                e = e_tiles[kt]
                t1 = t1_tiles.pop(kt)
                t2 = work.tile([128, 4, QB], bf16, tag="t2",
                               bufs=CFG.get("T2BUFS", 1), name=f"t2_{qb}_{kt}")
                nc.vector.tensor_add(t2[:], t1[:, 0, :, :], t1[:, 1, :, :])
                t3 = work.tile([128, 2, QB], bf16, tag="t3", bufs=1,
                               name=f"t3_{qb}_{kt}")
                eng3 = nc.gpsimd if CFG["T3"] == "pool" else nc.vector
                eng3.tensor_add(t3[:], t2[:, 0:2, :], t2[:, 2:4, :])
                zf = work.tile([128, QB], f32, tag="zf",
                               bufs=CFG.get("ZFBUFS", 2), name=f"zf{qb}_{kt}")
                engz = nc.gpsimd if CFG["ZF"] == "pool" else nc.vector
                engz.tensor_add(zf[:], t3[:, 0, :], t3[:, 1, :])
                rf = work.tile([128, QB], f32, tag="rf", bufs=1,
                               name=f"rf{qb}_{kt}")
                nc.vector.reciprocal_approx_fast(rf[:], zf[:])
                rb = work.tile([128, QB], bf16, tag="rb", bufs=1,
                               name=f"rb{qb}_{kt}")
                nc.vector.tensor_copy(rb[:], rf[:])
                nc.vector.tensor_mul(
                    e[:, :, :], e[:, :, :],
                    rb[:].unsqueeze(1).broadcast_to([128, H, QB]))
                if CFG.get("DEBUG") and qb == 0 and kt == 0:
                    nc.sync.dma_start(dbg["e0"],
                                      e.rearrange("p h q -> p (h q)"))

            def attnv(kt, qb=qb, out_ps=out_ps, e_tiles=e_tiles):
                e = e_tiles.pop(kt)
                # start/stop once per 2KB PSUM bank: start zeroes the whole
                # bank, so only the first matmul touching a bank may set it.
                for h in range(H):
                    for qc in range(2):
                        nc.tensor.matmul(
                            out_ps[:, qc, h * DK:(h + 1) * DK],
                            e[:, h, qc * 128:(qc + 1) * 128],
                            v[:, kt, h, :],
                            start=(kt == 0 and h % 8 == 0),
                            stop=(kt == NKT - 1 and h % 8 == 7),
                            skip_group_check=True)

            for kt in range(NKT):
                if qb == 0:
                    # spread projection emission: 2 k-chunks per kt of the
                    # NEXT 512-block, 2 q-chunks per kt over kt=1..4, one
                    # v-tile two k-tiles ahead.
                    blk = kt // 4 + 1
                    if blk < 4:
                        sub = kt % 4
                        kproj(blk, sub * 2, sub * 2 + 2)
                    if kt + 2 < NKT:
                        vproj(kt + 2)
                if qb == 1 and kt <= 3:
                    qproj(1, kt * 2, kt * 2 + 2)
                e = work.tile([128, H, QB], bf16, tag="e", bufs=CFG["EBUFS"],
                              name=f"e{qb}_{kt}")
                e_tiles[kt] = e
                # scores (4 heads = 2 chunk-pairs per psum tile) + exp.
                for hg in range(4):
                    ps = psum_s.tile([128, 1024], f32, tag="s",
                                     name=f"pss{qb}_{kt}_{hg}")
                    for cl in range(2):
                        c = hg * 2 + cl
                        nc.tensor.matmul(
                            ps[:, cl * 512:(cl + 1) * 512],
                            kT[:, c, kt * 128:(kt + 1) * 128],
                            qT2[:, c, :, q0:q0 + QB],
                            start=True, stop=True)
                    nc.scalar.activation(
                        e[:, hg * 4:(hg + 1) * 4, :], ps[:, :], AF.Exp)
                if CFG.get("STAGED", 1):
                    stageA(qb, kt)
                    if kt >= 1:
                        stageB(qb, kt - 1)
                else:
                    stageA(qb, kt)
                    stageB(qb, kt)
                if kt >= LAG:
                    attnv(kt - LAG)
                if kt == 1 and pending_evac:
                    pending_evac.pop()()
            if CFG.get("STAGED", 1):
                stageB(qb, NKT - 1)
            for kt in range(NKT - LAG, NKT):
                attnv(kt)
            pending_evac.append(lambda qb=qb, q0=q0, out_ps=out_ps:
                                emit_evac(qb, q0, out_ps))
        pending_evac.pop()()
            for qc in range(2):
                nc.scalar.activation(oq[:, qc, :], out_ps[:, qc, :], AF.Copy)
            tp = psum_s.tile([128, 16, 128], bf16, tag="s", name=f"tp{qb}")
            for qc in range(2):
                for cch in range(8):
                    nc.tensor.transpose(
                        tp[:, qc * 8 + cch, :],
                        oq[:, qc, cch * 128:(cch + 1) * 128], id_t[:])
            oT = work.tile([128, 2, 8, 128], bf16, tag="oT", bufs=1,
                           name=f"oT{qb}")
            nc.vector.tensor_copy(oT[:, 0, :, :], tp[:, 0:8, :])
            nc.vector.tensor_copy(oT[:, 1, :, :], tp[:, 8:16, :])
            if CFG.get("DEBUG") and qb == 0:
                nc.sync.dma_start(dbg["oq0"],
                                  oq.rearrange("p c f -> p (c f)"))
                nc.sync.dma_start(dbg["oT0"],
                                  oT.rearrange("p c t q -> p (c t q)"))
            for qc in range(2):
                yps = psum_s.tile([128, 1024], f32, tag="s",
                                  name=f"yps{qb}_{qc}")
                for fb in range(2):
                    for cch in range(8):
                        nc.tensor.matmul(
                            yps[:, fb * 512:(fb + 1) * 512],
                            oT[:, qc, cch, :],
                            wo[:, cch, fb * 512:(fb + 1) * 512],
                            start=(cch == 0), stop=False)
                    nc.tensor.matmul(
                        yps[:, fb * 512:(fb + 1) * 512], ones_t[:, :],
                        bo_t[:, fb * 512:(fb + 1) * 512],
                        start=False, stop=True)
                y = work.tile([128, 1024], f32, tag="y", bufs=1,
                              name=f"y{qb}_{qc}")
                nc.scalar.activation(y[:], yps[:], AF.Copy)
                nc.sync.dma_start(
                    out_d[q0 + qc * 128: q0 + (qc + 1) * 128, :], y[:])

        if CFG.get("DEBUG"):
            nc.sync.dma_start(dbg["kT"], kT.rearrange("p c s -> p (c s)"))
            nc.sync.dma_start(dbg["qT2"],
                              qT2.rearrange("p c t s -> p (c t s)"))
            nc.sync.dma_start(dbg["v"], v.rearrange("p t h k -> p (t h k)"))

    nc.compile()
    return nc


def _get_nc():
    if "nc" not in _CACHE:
        _CACHE["nc"] = _build_bass()
    return _CACHE["nc"]


def _make_in_maps(query, key, value, W_split, b_split, W_o, b_o):
    import ml_dtypes
    bf16 = ml_dtypes.bfloat16

    query = np.asarray(query, np.float32)
    key = np.asarray(key, np.float32)
    value = np.asarray(value, np.float32)
    W_split = np.asarray(W_split, np.float32)
    b_split = np.asarray(b_split, np.float32)
    W_o = np.asarray(W_o, np.float32)
    b_o = np.asarray(b_o, np.float32)

    wsT = np.ascontiguousarray(W_split.T.astype(bf16))
    woT = np.ascontiguousarray(W_o.T.astype(bf16))
    bq = np.ascontiguousarray((b_split / 8.0).reshape(8, 128).T)
    bk = np.ascontiguousarray(b_split.reshape(8, 128).T)
    bvb = np.ascontiguousarray(
        np.broadcast_to(b_split, (128, D)).astype(bf16))
    bo = np.zeros((32, D), bf16)
    bo[0] = b_o.astype(bf16)
    idm = np.eye(128, dtype=bf16)

    kTs = [np.ascontiguousarray(key[:, b, :].T.astype(bf16))
           for b in range(BATCH)]
    vTs = [np.ascontiguousarray(value[:, b, :].T.astype(bf16))
           for b in range(BATCH)]
    in_maps = []
    for d in range(NCORES):
        b, qc = d // 2, d % 2
        xqT = np.ascontiguousarray(
            query[qc * QCH:(qc + 1) * QCH, b, :].T.astype(bf16))
        in_maps.append({
            "xqT": xqT, "xkT": kTs[b], "xvT": vTs[b],
            "wsT": wsT, "woT": woT,
            "bq": bq, "bk": bk, "bvb": bvb, "bo": bo, "idm": idm,
        })
    return in_maps


def kernel_with_results(trace=False, **inputs):
    from concourse.bass_utils import run_bass_kernel_spmd

    nc = _get_nc()
    in_maps = _make_in_maps(**inputs)
    last_exc = None
    for _attempt in range(3):
        try:
            res = run_bass_kernel_spmd(nc, in_maps,
                                       core_ids=list(range(NCORES)),
                                       trace=trace)
            break
        except Exception as exc:  # rare transient device fault -> retry
            last_exc = exc
    else:
        raise last_exc
    out = np.empty((SEQ, BATCH, D), np.float32)
    for d in range(NCORES):
        b, qc = d // 2, d % 2
        out[qc * QCH:(qc + 1) * QCH, b, :] = res.results[d]["out"]
    return out, res


def kernel(**inputs):
    out, _ = kernel_with_results(trace=False, **inputs)
    return out
